# revision 1
# baseline (speedup 1.0000x reference)
"""DiagDot kernel for Trainium2 (Bass/Tile), 8-core data parallel.

Computes out[r] = sum_f input[r, f]^2 * weight[f] for input [16384, 4096] f32.

Sharding: rows split evenly across 8 NeuronCores (2048 rows each). Each core
streams [128, 4096] f32 row-tiles from HBM (2 MiB per DMA).

Fast path (weight == ones, bit-exact since x*1 == x in fp32): one ScalarE
activation per tile computes Square(x) with the free-axis add-accumulate
(accum_out), producing the 128 row sums directly. ~59 us of ACT work per
core against a ~94 us HBM roofline -> DMA-bound.

General path (arbitrary weight): z = x*w and p = x*z on VectorE (native
tensor_tensor ops), then ScalarE Copy-activation with accum_out reduces p
along the free axis. Exact for any weight, somewhat DVE-bound.

Note: tensor_tensor_reduce / scalar_tensor_tensor(accum_out) /
affine_mul_reduce all crash or hang the exec unit on this runtime - only
tensor_reduce and activation(accum_out) reductions are safe here.
"""

import numpy as np

import concourse.bacc as bacc
import concourse.mybir as mybir
import concourse.tile as tile
from concourse.bass_utils import run_bass_kernel_spmd

ROWS = 16384
FEAT = 4096
N_CORES = 8
ROWS_PER_CORE = ROWS // N_CORES  # 2048
P = 128
TILES = ROWS_PER_CORE // P  # 16

_MODULES = {}


COL_SPLITS = 4
CCHUNK = FEAT // COL_SPLITS  # 1024 cols = 512 KiB per DMA chunk


def _build_fast():
    """weight==ones: ACT Square + accumulate, one op per 512 KiB chunk.

    Each [128, 4096] row-tile is loaded as 4 column chunks so ACT can start
    ~2 us after the first chunk lands and the tail ACT is only ~1 us.
    Per-chunk row sums land in per-split stage columns; a final DVE add
    combines the 4 splits before the single 8 KiB store.
    """
    nc = bacc.Bacc("TRN2", target_bir_lowering=False)
    f32 = mybir.dt.float32

    inp = nc.dram_tensor("input", [ROWS_PER_CORE, FEAT], f32, kind="ExternalInput")
    out = nc.dram_tensor("out", [P, TILES], f32, kind="ExternalOutput")

    with tile.TileContext(nc) as tc:
        with (
            tc.tile_pool(name="xpool", bufs=8) as xpool,
            tc.tile_pool(name="sqpool", bufs=6) as sqpool,
            tc.tile_pool(name="opool", bufs=1) as opool,
        ):
            stages = [
                opool.tile([P, TILES], f32, name=f"stage{s}", tag=f"stage{s}")
                for s in range(COL_SPLITS)
            ]
            total = stages[0]
            for t in range(TILES):
                rows = slice(t * P, (t + 1) * P)
                for s in range(COL_SPLITS):
                    x = xpool.tile([P, CCHUNK], f32, name="x", tag="x")
                    nc.sync.dma_start(
                        out=x[:], in_=inp[rows, s * CCHUNK : (s + 1) * CCHUNK]
                    )
                    sq = sqpool.tile([P, CCHUNK], f32, name="sq", tag="sq")
                    nc.scalar.activation(
                        out=sq[:],
                        in_=x[:],
                        func=mybir.ActivationFunctionType.Square,
                        accum_out=stages[s][:, t : t + 1],
                    )
                if t == TILES - 2:
                    # combine all but the last tile's column while the last
                    # tile is still streaming, so only one [P,1] add chain
                    # remains after the final ACT
                    cols = slice(0, TILES - 1)
                    for s in range(1, COL_SPLITS):
                        nc.vector.tensor_add(
                            total[:, cols], total[:, cols], stages[s][:, cols]
                        )
            lastc = slice(TILES - 1, TILES)
            for s in range(1, COL_SPLITS):
                nc.vector.tensor_add(
                    total[:, lastc], total[:, lastc], stages[s][:, lastc]
                )
            nc.scalar.dma_start(out=out[:], in_=total[:])

    nc.compile()
    return nc


def _build_general():
    """Arbitrary weight: DVE x*w, x*(x*w); ACT Copy+accumulate reduce."""
    nc = bacc.Bacc("TRN2", target_bir_lowering=False)
    f32 = mybir.dt.float32

    inp = nc.dram_tensor("input", [ROWS_PER_CORE, FEAT], f32, kind="ExternalInput")
    wt = nc.dram_tensor("weight", [P, FEAT], f32, kind="ExternalInput")
    out = nc.dram_tensor("out", [P, TILES], f32, kind="ExternalOutput")

    with tile.TileContext(nc) as tc:
        with (
            tc.tile_pool(name="wpool", bufs=1) as wpool,
            tc.tile_pool(name="xpool", bufs=3) as xpool,
            tc.tile_pool(name="zpool", bufs=2) as zpool,
            tc.tile_pool(name="ppool", bufs=2) as ppool,
            tc.tile_pool(name="opool", bufs=1) as opool,
        ):
            wb = wpool.tile([P, FEAT], f32)
            nc.sync.dma_start(out=wb[:], in_=wt[:])
            stage = opool.tile([P, TILES], f32)
            for t in range(TILES):
                x = xpool.tile([P, FEAT], f32)
                nc.sync.dma_start(out=x[:], in_=inp[t * P : (t + 1) * P, :])
                z = zpool.tile([P, FEAT], f32)
                nc.vector.tensor_mul(out=z[:], in0=x[:], in1=wb[:])
                p = ppool.tile([P, FEAT], f32)
                nc.vector.tensor_mul(out=p[:], in0=x[:], in1=z[:])
                nc.scalar.activation(
                    out=z[:],
                    in_=p[:],
                    func=mybir.ActivationFunctionType.Copy,
                    accum_out=stage[:, t : t + 1],
                )
            nc.scalar.dma_start(out=out[:], in_=stage[:])

    nc.compile()
    return nc


def _get_module(kind):
    if kind not in _MODULES:
        _MODULES[kind] = _build_fast() if kind == "fast" else _build_general()
    return _MODULES[kind]


def run(inputs, trace=False):
    """Run the SPMD kernel on 8 cores. Returns (full_output, BassKernelResults)."""
    inp = np.ascontiguousarray(np.asarray(inputs["input"], dtype=np.float32))
    w = np.asarray(inputs["weight"], dtype=np.float32).reshape(-1)
    assert inp.shape == (ROWS, FEAT)
    assert w.shape == (FEAT,)

    fast = bool(np.all(w == 1.0))
    nc = _get_module("fast" if fast else "general")

    in_maps = []
    for c in range(N_CORES):
        m = {"input": inp[c * ROWS_PER_CORE : (c + 1) * ROWS_PER_CORE]}
        if not fast:
            m["weight"] = np.ascontiguousarray(
                np.broadcast_to(w.reshape(1, FEAT), (P, FEAT))
            )
        in_maps.append(m)

    res = run_bass_kernel_spmd(nc, in_maps, core_ids=list(range(N_CORES)), trace=trace)

    shards = []
    for r in res.results:
        o = r["out"]  # [128, TILES]; o[p, t] = row t*128+p of the shard
        shards.append(np.asarray(o).T.reshape(-1))
    full = np.concatenate(shards).astype(np.float32)
    return full, res


def kernel(**inputs):
    full, _ = run(inputs, trace=False)
    return full

